# revision 28
# baseline (speedup 1.0000x reference)
"""Trainium2 Bass kernel for nn_BetterAttendCompareAggregate — sparse packed.

Math (per batch b, q_b/p_b = [L, D] slices, M = WF^T WF):
    e = q_b @ M @ p_b^T restricted to valid rows/cols (qm/pm == 1); masked
    rows/cols of the reference softmaxes contribute exactly nothing to the
    output, so we never compute them:
      out[b] = g1a.sum_all(q) + g2a.sum_all(p)              (host, exact)
             + sum_j cj[j] * (p_j . g1b)                    (cj from device)
             + sum_l ci[l] * (q_l . g2b)                    (ci from device)
    with cj[j] = sum_l exp(e_lj) qm_l/den_l, ci[l] = sum_j exp(e_lj) pm_j/S_j,
    den/S the row/col sums of exp(e - SHIFT) (shift cancels in the ratios,
    like the reference's max subtraction; eps terms are ~1e-8 relative).

Device work per core = A = M @ q_packed (valid q cols of its 8 batches,
~1053 of 2048 cols) then per batch: e = A^T p_valid, exp straight out of
PSUM (activation bias = -SHIFT; zero padding self-masks as exp(-100)~=0),
PE transposes for the second softmax orientation, and two rank-1 colsum
matmuls producing cj/ci. Everything else (the folded Compare/Aggregate
linear layer) is tiny host pre/post-processing.

Batches are assigned to (core, slot) by rank-grouped sort of the valid
counts so the compile-time per-slot sizes are data-exact to within ~2%,
and each slot picks the e-matmul orientation (q- or p-stationary) that
minimizes PE columns. All matmul chains in bf16 (rel-err budget 2e-2,
measured ~7e-3, dominated by bf16 noise on e averaging out in colsums).
"""

from collections import deque

import numpy as np

from concourse import bacc, mybir, tile
from concourse.bass_utils import run_bass_kernel_spmd

P = 128
D = 1024
KC = D // P
MC = D // P
L = 256
B = 64
NCORES = 8
NB = B // NCORES          # slots (batches) per core
SHIFT = 100.0             # exp(e - SHIFT) keeps fp32 range; cancels in ratios
EPS = 1e-37
F32 = mybir.dt.float32
BD = mybir.dt.bfloat16

MAX_AG = 512              # psum bank limit for fp32 A accumulation


def _cdiv(a, b):
    return -(-a // b)


class _Cfg:
    """Compile-time packing config derived from the actual masks."""

    def __init__(self, qm, pm):
        nq = qm.sum(1).astype(np.int64)
        npv = pm.sum(1).astype(np.int64)
        best = None
        for key in (nq, npv, nq + npv, np.maximum(nq, npv)):
            order = np.argsort(-key, kind="stable")
            cost = self._model(nq, npv, order)
            if best is None or cost < best[0]:
                best = (cost, order)
        self.order = best[1]
        self.nq, self.npv = nq, npv
        self.LVq, self.LVp, self.orient = [], [], []
        for s in range(NB):
            grp = self.order[s * NCORES:(s + 1) * NCORES]
            # multiple of 4 keeps every PSUM slice offset 4B-aligned
            lq = -(-int(nq[grp].max()) // 4) * 4
            lp = -(-int(npv[grp].max()) // 4) * 4
            self.LVq.append(lq)
            self.LVp.append(lp)
            self.orient.append(self._best_orient(lq, lp)[1])
        self.Oq = np.concatenate([[0], np.cumsum(self.LVq)])
        self.Op = np.concatenate([[0], np.cumsum(self.LVp)])
        self.NVq = int(self.Oq[-1])
        self.NVp = int(self.Op[-1])
        # A column groups: consecutive slots, total cols <= 512 (psum bank)
        self.groups = []      # (col_offset, ncols, [slots])
        cur = []
        for s in range(NB):
            w = sum(self.LVq[t] for t in cur)
            if cur and w + self.LVq[s] > MAX_AG:
                self.groups.append((int(self.Oq[cur[0]]), w, cur))
                cur = []
            cur.append(s)
        w = sum(self.LVq[t] for t in cur)
        self.groups.append((int(self.Oq[cur[0]]), w, cur))
        # rv (validity weight) column offsets: q chunks then p chunks per slot
        self.rvq, self.rvp = [], []
        o = 0
        for s in range(NB):
            self.rvq.append(o)
            o += _cdiv(self.LVq[s], P)
            self.rvp.append(o)
            o += _cdiv(self.LVp[s], P)
        self.nrv = o

    @staticmethod
    def _best_orient(lq, lp):
        best = None
        for R, lr, lc in (("q", lq, lp), ("p", lp, lq)):
            cr, cc = _cdiv(lr, P), _cdiv(lc, P)
            cols = cr * KC * lc + cr * lc + cr * lc + cc * lr
            instr = cr * KC + cr * cc + cr + cc
            c = cols * 0.42 + instr * 40.0
            if best is None or c < best[0]:
                best = (c, R)
        return best

    @classmethod
    def _model(cls, nq, npv, order):
        cost = 0.0
        sum_lq = 0
        for s in range(NB):
            grp = order[s * NCORES:(s + 1) * NCORES]
            lq, lp = int(nq[grp].max()), int(npv[grp].max())
            sum_lq += lq
            cost += cls._best_orient(lq, lp)[0]
        cost += sum_lq * MC * KC * 0.42
        return cost

    def core_batches(self, c):
        return [int(self.order[s * NCORES + c]) for s in range(NB)]


def _body(tc, qG, pS, Mt, idt, rv, out, cfg):
    nc = tc.nc
    ACT = mybir.ActivationFunctionType
    OP = mybir.AluOpType
    NVq, NVp = cfg.NVq, cfg.NVp

    with (
        tc.tile_pool(name="singles", bufs=1) as singles,
        tc.tile_pool(name="xp", bufs=3) as xpool,
        tc.tile_pool(name="xtp", bufs=3) as xtpool,
        tc.tile_pool(name="small", bufs=16) as small,
        tc.tile_pool(name="ps_a", bufs=3, space="PSUM") as ps_a,
        tc.tile_pool(name="ps_e", bufs=2, space="PSUM") as ps_e,
        tc.tile_pool(name="ps_ts", bufs=3, space="PSUM") as ps_ts,
    ):
        # ---- input DMAs.  gpsimd: M head + per-slot p blocks (in slot
        # order); scalar: M tail (frees scalar before its exp/copy work);
        # sync: first q group + consts; vector: remaining q groups. ----
        # SBUF write tracking is tile-granular: a reader waits on EVERY
        # DMA into its tile, so each DMA gets its own tile.  HBM bandwidth
        # is the startup constraint (~7MB of inputs vs ~360GB/s): only the
        # critical-path tiles (M head + first q k-chunks) are fetched
        # upfront; everything else is staged behind compute via dummy
        # writes so its transfer starts only after the critical data lands
        # (DMA channels serialize in emission order).
        M0 = singles.tile([P, 2, KC, P], BD)
        M1 = singles.tile([P, 3, KC, P], BD)
        M2 = singles.tile([P, 3, KC, P], BD)

        def Mst(m, k):
            if m < 2:
                return M0[:, m, k]
            if m < 5:
                return M1[:, m - 2, k]
            return M2[:, m - 5, k]

        qt = {}
        g0nc = cfg.groups[0][1]
        for kh in range(4):
            qt[(0, kh)] = singles.tile([P, 2 * g0nc], BD, name=f"q0h{kh}")
        for gi in range(1, len(cfg.groups)):
            qt[gi] = singles.tile([P, KC * cfg.groups[gi][1]], BD,
                                  name=f"qg{gi}")

        def qmov(gi, k):
            gnc = cfg.groups[gi][1]
            if gi == 0:
                return qt[(0, k // 2)][:, (k % 2) * gnc:(k % 2 + 1) * gnc]
            return qt[gi][:, k * gnc:(k + 1) * gnc]

        pt = [
            singles.tile(
                [P, KC * (int(cfg.Op[s + 1]) - int(cfg.Op[s]))], BD,
                name=f"ps{s}")
            for s in range(NB)
        ]
        idt_sb = singles.tile([P, P], BD)
        rv_sb = singles.tile([P, cfg.nrv], F32)

        def dma_q0h(kh, eng):
            a = KC * cfg.groups[0][0] + 2 * kh * g0nc
            eng.dma_start(qt[(0, kh)][:], qG[:, a:a + 2 * g0nc])

        # upfront: exactly what the first A pair touches
        nc.gpsimd.dma_start(M0[:], Mt[:, 0:2])
        dma_q0h(0, nc.sync)
        dma_q0h(1, nc.scalar)

        # deferred DMAs, staged after the evac of successive A pairs
        deferred = deque()
        for gi in range(1, len(cfg.groups)):
            goff, gnc, _ = cfg.groups[gi]
            deferred.append((qt[gi], qG[:, KC * goff:KC * (goff + gnc)]))
        for s in range(NB):
            a, bnd = KC * int(cfg.Op[s]), KC * int(cfg.Op[s + 1])
            deferred.append((pt[s], pS[:, a:bnd]))

        def stage(src_ap, n, engs):
            """Issue up to n deferred DMAs gated on src_ap being written."""
            for i in range(min(n, len(deferred))):
                dst, dsrc = deferred.popleft()
                nc.vector.tensor_copy(dst[0:1, 0:1], src_ap)
                engs[i % len(engs)].dma_start(dst[:], dsrc)

        A_sb = [
            singles.tile([P, MC, gnc], BD, name=f"A{gi}")
            for gi, (_, gnc, _) in enumerate(cfg.groups)
        ]
        outB = singles.tile([1, NVp + NVq], F32)
        shb = singles.tile([P, 1], F32)
        nc.vector.memset(shb[:], -SHIFT)

        # per-slot intermediates kept across conveyor ticks
        Xs = [None] * NB
        XTs = [None] * NB
        dRs = [None] * NB
        recRs = [None] * NB
        recCs = [None] * NB

        slot_g = {}
        for gi, (goff, _, gsl) in enumerate(cfg.groups):
            for s in gsl:
                slot_g[s] = (gi, int(cfg.Oq[s]) - goff)

        def sdims(s):
            if cfg.orient[s] == "q":
                return cfg.LVq[s], cfg.LVp[s]
            return cfg.LVp[s], cfg.LVq[s]

        def _emit_rec(rec, dd, nch, ll, rvo):
            """rec = rv / (dd + eps), per 128-chunk (bf16 softmax weights)."""
            for c in range(nch):
                w = min(P, ll - c * P)
                nc.vector.tensor_scalar_add(
                    rec[0:w, c:c + 1], dd[0:w, c:c + 1], EPS)
                with nc.allow_low_precision(reason="bf16 softmax wts"):
                    nc.vector.reciprocal(rec[0:w, c:c + 1], rec[0:w, c:c + 1])
                nc.vector.tensor_tensor(
                    rec[0:w, c:c + 1], rec[0:w, c:c + 1],
                    rv_sb[0:w, rvo + c:rvo + c + 1], OP.mult)

        def emit_S1(s):
            """e matmuls + exp/rowsum for slot s."""
            lr, lc = sdims(s)
            CR = _cdiv(lr, P)
            gi, loq = slot_g[s]
            Ag = A_sb[gi]
            psE = ps_e.tile([P, CR, lc], F32, tag="e", name=f"psE{s}")
            for c in range(CR):
                w = min(P, lr - c * P)
                for k in range(KC):
                    if cfg.orient[s] == "q":
                        stat = Ag[:, k, loq + c * P:loq + c * P + w]
                        mv = pt[s][:, k * lc:(k + 1) * lc]
                    else:
                        base = k * lr
                        stat = pt[s][:, base + c * P:base + c * P + w]
                        mv = Ag[:, k, loq:loq + lc]
                    nc.tensor.matmul(
                        psE[0:w, c, :], stat, mv,
                        start=(k == 0), stop=(k == KC - 1),
                    )
            X = xpool.tile([P, CR, lc], BD, tag="x", name=f"x{s}")
            dR = small.tile([P, CR], F32, tag="dR", name=f"dR{s}")
            for c in range(CR):
                w = min(P, lr - c * P)
                nc.scalar.activation(
                    X[0:w, c, :], psE[0:w, c, :], ACT.Exp,
                    bias=shb[0:w, 0:1], accum_out=dR[0:w, c:c + 1],
                )
            # recR only needs dR: compute it now so the S6 colsum matmuls
            # a tick later never wait on the DVE chain
            rvR = cfg.rvq[s] if cfg.orient[s] == "q" else cfg.rvp[s]
            recR = small.tile([P, CR], BD, tag="recR", name=f"recR{s}")
            _emit_rec(recR, dR, CR, lr, rvR)
            Xs[s], dRs[s], recRs[s] = X, dR, recR

        def emit_S3(s):
            """transposes + colsum evac + reciprocal weights for slot s."""
            lr, lc = sdims(s)
            CR, CC = _cdiv(lr, P), _cdiv(lc, P)
            X, dR = Xs[s], dRs[s]
            psT = ps_ts.tile([P, CC, lr], BD, tag="ts", name=f"psT{s}")
            for c2 in range(CC):
                w2 = min(P, lc - c2 * P)
                for c in range(CR):
                    w = min(P, lr - c * P)
                    nc.tensor.transpose(
                        psT[0:w2, c2, c * P:c * P + w],
                        X[0:w, c, c2 * P:c2 * P + w2],
                        idt_sb[0:w, 0:w],
                    )
            XT = xtpool.tile([P, CC, lr], BD, tag="xt", name=f"xt{s}")
            dC = small.tile([P, CC], F32, tag="dC", name=f"dC{s}")
            for c2 in range(CC):
                w2 = min(P, lc - c2 * P)
                nc.scalar.activation(
                    XT[0:w2, c2, :], psT[0:w2, c2, :], ACT.Copy,
                    accum_out=dC[0:w2, c2:c2 + 1],
                )
            rvC = cfg.rvp[s] if cfg.orient[s] == "q" else cfg.rvq[s]
            recC = small.tile([P, CC], BD, tag="recC", name=f"recC{s}")
            _emit_rec(recC, dC, CC, lc, rvC)
            # the X-side colsum only needs X and recR, both ready: emit it
            # here so the final slot's exposed chain is one matmul shorter
            recR = recRs[s]
            psCc = ps_ts.tile([1, lc], F32, tag="ts", name=f"cc{s}")
            for c in range(CR):
                w = min(P, lr - c * P)
                nc.tensor.matmul(
                    psCc[:], recR[0:w, c:c + 1], X[0:w, c, :],
                    start=(c == 0), stop=(c == CR - 1),
                )
            oq, op = int(cfg.Oq[s]), int(cfg.Op[s])
            if cfg.orient[s] == "q":
                nc.vector.tensor_copy(outB[0:1, op:op + lc], psCc[:])
            else:
                nc.vector.tensor_copy(outB[0:1, NVp + oq:NVp + oq + lc],
                                      psCc[:])
            XTs[s], recCs[s] = XT, recC

        def emit_S6(s):
            """XT-side colsum matmul + output row evac for slot s."""
            lr, lc = sdims(s)
            CC = _cdiv(lc, P)
            XT, recC = XTs[s], recCs[s]
            psCr = ps_ts.tile([1, lr], F32, tag="ts", name=f"cr{s}")
            for c2 in range(CC):
                w2 = min(P, lc - c2 * P)
                nc.tensor.matmul(
                    psCr[:], recC[0:w2, c2:c2 + 1], XT[0:w2, c2, :],
                    start=(c2 == 0), stop=(c2 == CC - 1),
                )
            oq, op = int(cfg.Oq[s]), int(cfg.Op[s])
            if cfg.orient[s] == "q":
                nc.vector.tensor_copy(outB[0:1, NVp + oq:NVp + oq + lr],
                                      psCr[:])
            else:
                nc.vector.tensor_copy(outB[0:1, op:op + lr], psCr[:])
            Xs[s] = XTs[s] = None

        # ---- warm the PE clock (pstate ramps over ~3us of sustained use)
        # while the first q group + M head DMA in. ----
        wrm = singles.tile([P, 2 * P], BD)
        nc.vector.memset(wrm[:], 0.0)
        ps_w = ps_e.tile([P, 2 * P], F32, tag="e", name="warm")
        for w in range(13):
            nc.tensor.matmul(ps_w[:, 0:2 * P], wrm[:, 0:P], wrm[:],
                             start=True, stop=True)
        # wave 1, gated on the first q chunk landing (~11.5us): next q
        # chunk and the mid M block get the full bandwidth next
        nc.vector.tensor_copy(qt[(0, 2)][0:1, 0:1], qt[(0, 0)][0:1, 1:2])
        nc.vector.tensor_copy(M1[0:1, 0, 0, 0:1], qt[(0, 0)][0:1, 1:2])
        dma_q0h(2, nc.sync)
        nc.scalar.dma_start(M1[:], Mt[:, 2:5])
        # wave 2, gated on wave 1's q chunk landing: the rest
        nc.vector.tensor_copy(qt[(0, 3)][0:1, 0:1], qt[(0, 2)][0:1, 1:2])
        nc.vector.tensor_copy(M2[0:1, 0, 0, 0:1], qt[(0, 2)][0:1, 1:2])
        dma_q0h(3, nc.scalar)
        nc.gpsimd.dma_start(M2[:], Mt[:, 5:8])
        nc.sync.dma_start(idt_sb[:], idt[:])
        nc.sync.dma_start(rv_sb[:], rv[:])
        if len(cfg.groups) > 1:
            # group 1's q is large (~1MB); wave 3 on the previous chunk
            # landing so it beats the A group transition comfortably
            g1t = qt[1]
            goff, gnc, _ = cfg.groups[1]
            nc.vector.tensor_copy(g1t[0:1, 0:1], qt[(0, 3)][0:1, 1:2])
            nc.sync.dma_start(g1t[:], qG[:, KC * goff:KC * (goff + gnc)])
            deferred.popleft()

        def emit_A_pair(gi, mp, mid=None):
            gnc = cfg.groups[gi][1]
            psA2 = [
                ps_a.tile([P, gnc], F32, tag="psA", name=f"psA{gi}_{mp + j}")
                for j in range(2)
            ]
            for k in range(KC):
                if k == KC // 2 and mid is not None:
                    mid()   # inject conveyor work mid-pair
                mvq = qmov(gi, k)
                for j in range(2):
                    nc.tensor.matmul(
                        psA2[j][:], Mst(mp + j, k), mvq,
                        start=(k == 0), stop=(k == KC - 1),
                    )
            nc.vector.tensor_copy(A_sb[gi][:, mp, :], psA2[0][:])
            nc.scalar.copy(A_sb[gi][:, mp + 1, :], psA2[1][:])
            stage(A_sb[gi][0:1, mp, 0:1], 3, (nc.gpsimd, nc.sync))

        # ---- conveyor: A pairs feed PE; slot softmax stages slot into the
        # stream one tick behind their scalar/DVE producers. ----
        s1q, s3q, s6q = deque(), deque(), deque()

        def tick():
            if s6q:
                emit_S6(s6q.popleft())
            if s3q:
                s = s3q.popleft()
                emit_S3(s)
                s6q.append(s)
            if s1q:
                s = s1q.popleft()
                emit_S1(s)
                s3q.append(s)

        for gi, (_, _, gslots) in enumerate(cfg.groups):
            for mp in range(0, MC, 2):
                emit_A_pair(gi, mp, mid=tick if gi > 0 else None)
                if gi > 0:
                    tick()
            s1q.extend(gslots)
        if s1q:
            s = s1q.popleft()
            emit_S1(s)
            s3q.append(s)
        while s1q or s3q or s6q:
            tick()

        nc.sync.dma_start(out[:], outB[:])


_PROGRAM = None
_CFG = None
_HOST = None


def build_program(cfg):
    nc = bacc.Bacc(
        "TRN2", target_bir_lowering=False, debug=False, num_devices=NCORES
    )
    qG = nc.dram_tensor("qG", [P, KC * cfg.NVq], BD, kind="ExternalInput").ap()
    pS = nc.dram_tensor("pS", [P, KC * cfg.NVp], BD, kind="ExternalInput").ap()
    Mt = nc.dram_tensor("M", [P, MC, KC, P], BD, kind="ExternalInput").ap()
    idt = nc.dram_tensor("idt", [P, P], BD, kind="ExternalInput").ap()
    rv = nc.dram_tensor("rv", [P, cfg.nrv], F32, kind="ExternalInput").ap()
    out = nc.dram_tensor("out", [1, cfg.NVp + cfg.NVq], F32,
                         kind="ExternalOutput").ap()
    with tile.TileContext(nc) as tc:
        _body(tc, qG, pS, Mt, idt, rv, out, cfg)
    nc.compile()
    return nc


def get_program():
    global _PROGRAM
    if _PROGRAM is None:
        _PROGRAM = build_program(_CFG)
    return _PROGRAM


def _pack_kmajor(cols):
    """[D, n] fp array -> [P, KC*n] bf16 image (k-major blocks)."""
    import ml_dtypes
    n = cols.shape[1]
    return np.ascontiguousarray(
        cols.reshape(KC, P, n).transpose(1, 0, 2).reshape(P, KC * n)
    ).astype(ml_dtypes.bfloat16)


def make_in_maps(q, p, qm, pm, WF, WG, WH):
    import ml_dtypes
    global _CFG, _HOST, _PROGRAM
    bf16 = ml_dtypes.bfloat16
    q, p = np.asarray(q), np.asarray(p)
    qm, pm = np.asarray(qm), np.asarray(pm)
    WF, WG, WH = np.asarray(WF), np.asarray(WG), np.asarray(WH)

    cfg = _Cfg(qm, pm)
    if _CFG is None or cfg.__dict__.keys() != _CFG.__dict__.keys() or any(
        not np.array_equal(cfg.__dict__[k], _CFG.__dict__[k])
        for k in ("order", "LVq", "LVp", "orient")
    ):
        _CFG, _PROGRAM = cfg, None

    WF64 = WF.astype(np.float64)
    M = (WF64.T @ WF64).astype(np.float32)
    M_img = np.ascontiguousarray(
        M.reshape(KC, P, MC, P).transpose(1, 2, 0, 3)).astype(bf16)
    idt_img = np.eye(P, dtype=np.float32).astype(bf16)

    WGT = WG.astype(np.float64).T
    g1 = WGT @ WH[0, :D].astype(np.float64)
    g2 = WGT @ WH[0, D:].astype(np.float64)
    base = (q.sum(0).astype(np.float64) @ g1[:D]
            + p.sum(0).astype(np.float64) @ g2[:D])       # [B]

    cfg = _CFG
    in_maps = []
    dqs, dps = {}, {}
    for c in range(NCORES):
        qcols = np.zeros((D, cfg.NVq), np.float32)
        pcols = np.zeros((D, cfg.NVp), np.float32)
        rvimg = np.zeros((P, cfg.nrv), np.float32)
        for s in range(NB):
            b = int(cfg.order[s * NCORES + c])
            qv = q[:, b, :][qm[b] == 1].T               # [D, nq_b]
            pv = p[:, b, :][pm[b] == 1].T
            nqb, npb = qv.shape[1], pv.shape[1]
            qcols[:, cfg.Oq[s]:cfg.Oq[s] + nqb] = qv
            pcols[:, cfg.Op[s]:cfg.Op[s] + npb] = pv
            dqs[b] = qv.T.astype(np.float64) @ g2[D:]   # [nq_b]
            dps[b] = pv.T.astype(np.float64) @ g1[D:]
            for nn, off in ((nqb, cfg.rvq[s]), (npb, cfg.rvp[s])):
                for ch in range(_cdiv(nn, P)):
                    w = min(P, nn - ch * P)
                    rvimg[0:w, off + ch] = 1.0
        # p image must be slot-k-major (per-slot contiguous blocks)
        pimg = np.empty((P, KC * cfg.NVp), bf16)
        for s in range(NB):
            a, bnd = int(cfg.Op[s]), int(cfg.Op[s + 1])
            pimg[:, KC * a:KC * bnd] = _pack_kmajor(pcols[:, a:bnd])
        # q image group-k-major
        qimg = np.empty((P, KC * cfg.NVq), bf16)
        for goff, gnc, _ in cfg.groups:
            qimg[:, KC * goff:KC * (goff + gnc)] = _pack_kmajor(
                qcols[:, goff:goff + gnc])
        in_maps.append({
            "qG": qimg, "pS": pimg, "M": M_img, "idt": idt_img, "rv": rvimg,
        })
    _HOST = {"base": base, "dq": dqs, "dp": dps}
    return in_maps


def core_batches(c):
    return _CFG.core_batches(c)


def assemble_core(out_arr, c):
    """Device out tensor [1, NVp+NVq] for core c -> [NB] output values."""
    cfg, host = _CFG, _HOST
    vals = np.zeros(NB, np.float64)
    for s in range(NB):
        b = int(cfg.order[s * NCORES + c])
        cj = out_arr[0, cfg.Op[s]:cfg.Op[s] + len(host["dp"][b])]
        ci = out_arr[0, cfg.NVp + cfg.Oq[s]:
                     cfg.NVp + cfg.Oq[s] + len(host["dq"][b])]
        vals[s] = (host["base"][b]
                   + cj.astype(np.float64) @ host["dp"][b]
                   + ci.astype(np.float64) @ host["dq"][b])
    return vals


def assemble_out(res):
    full = np.zeros((B, 1), np.float32)
    for c in range(NCORES):
        vals = assemble_core(np.asarray(res.results[c]["out"]), c)
        for s in range(NB):
            full[int(_CFG.order[s * NCORES + c]), 0] = vals[s]
    return full


def install_profile_hook():
    """Provide antenv.axon_hooks if the image lacks it (NTFF profiling)."""
    import sys
    import types

    try:
        from antenv.axon_hooks import get_axon_ntff_profile_hook  # noqa: F401
        return True
    except ImportError:
        pass
    try:
        from trn_agent_boot.trn_boot import _ntff_profile_via_ctypes

        hook = _ntff_profile_via_ctypes("/opt/axon/libaxon_pjrt.so")
        if hook is None:
            return False
        mod = types.ModuleType("antenv.axon_hooks")
        mod._hook = hook
        mod.get_axon_ntff_profile_hook = lambda: mod._hook

        def _set(h):
            mod._hook = h

        mod.set_axon_ntff_profile_hook = _set
        import antenv

        antenv.axon_hooks = mod
        sys.modules["antenv.axon_hooks"] = mod
        return True
    except Exception as e:  # pragma: no cover
        print(f"install_profile_hook failed: {e}")
        return False


def run(in_maps, trace=False, **kwargs):
    nc = get_program()
    if trace:
        install_profile_hook()
    return run_bass_kernel_spmd(
        nc, in_maps, core_ids=list(range(NCORES)), trace=trace, **kwargs
    )


def kernel(q, p, qm, pm, WF, WG, WH):
    in_maps = make_in_maps(q, p, qm, pm, WF, WG, WH)
    res = run(in_maps, trace=False)
    return assemble_out(res)
